# revision 22
# baseline (speedup 1.0000x reference)
"""Causal self-attention Trainium2 Bass kernel (v3).

Problem: B=4, T=2048, DIM=1024, H=16 heads, head_dim=64 (fp32).
  qkv = x @ w_qkv.T ; per-head causal softmax(q k^T / 8) v ; out @ w_out.T

Sharding (8 cores): core c -> (batch b = c//2, head-group g = c%2 of 8 heads).
Each core computes a partial output y_partial = attn_out_g @ w_out[:, g]^T
for its batch; host sums the two head-group partials per batch.

Structure (per core):
  stage1 (per 512-token chunk): QKV projection; q/k stored head-dim-major
    (qt/kt [128, T], 2 heads per tile), v token-major with a ones column
    per head ([128, 8*65]) so PV also emits the softmax denominator row.
  attention: one flat software pipeline over (head-pair, ktile).  The
    scores matmul stream (2 heads quad-packed via tile_position, K=64)
    runs one iteration ahead of exp+PV.  exp is split BY HEAD: head A on
    ScalarE (native Exp), head B usually on DVE (Schraudolph int16 trick)
    so both halves compute in parallel every iteration and PV never waits
    long.  Causal triangle trimming at 128-col granularity on the 4
    diagonal ktiles (scores/exp/PV only cover q >= j*128).  Diagonal-block
    masking: GpSimd 0/1-triangle multiply (ScalarE halves) or fused into
    the Schraudolph add constant (DVE halves).
  divide: ot PSUM is immediately evacuated to SBUF (frees the PSUM bank in
    ~0.6us instead of holding it through the ~5us divide chain), then
    reciprocal + DRAM-broadcast of 1/den + multiply produce aot.
  stage3: out-projection of the finished chunk.

Startup: PE warmup matmuls (HAM warm by the time stage1 starts) and the
input DMAs split across the two hardware DGE queues (sync + scalar).
"""

import contextlib

import numpy as np
import ml_dtypes

import concourse.bass as bass
import concourse.mybir as mybir
import concourse.tile as tile
from concourse import bacc
from concourse.bass_utils import run_bass_kernel_spmd

B, T, DIM = 4, 2048, 1024
NUM_HEADS, HEAD_DIM = 16, 64
INNER = NUM_HEADS * HEAD_DIM
SCALE = HEAD_DIM ** -0.5

N_CORES = 8
HEADS_PER_CORE = 8
HG = HEADS_PER_CORE * HEAD_DIM  # 512 = inner slice per core
NCH = T // 512                  # 4 token chunks
KT_PER_CH = 4                   # 128-ktok tiles per 512 chunk

F32R = mybir.dt.float32r
F32 = mybir.dt.float32
BF16 = mybir.dt.bfloat16
I16 = mybir.dt.int16

# Schraudolph exp in bf16-bits-as-int16: exp(s*SCALE) ~= bitcast_bf16(
#   int16(s*SCH_A + SCH_B)).  -5.5 centers the mantissa-interp error (+-3%).
LOG2E = 1.4426950408889634
SCH_A = float(SCALE * LOG2E * 128.0)
SCH_B = 127.0 * 128.0 - 5.5
# masked: v = s*SCH_A + SCH_BKILL stays in [~600, ~3500] for any plausible
# score s, so the bf16 bit pattern is a positive subnormal-exponent value
# ~2^-100 (never a negative int16, whose bit pattern would be a bf16 NaN).
SCH_BKILL = 2048.0

# Head-B exp engine per attention iteration: True -> DVE Schraudolph,
# False -> merged into head A's ScalarE activation.
DVE_B_PAT = (True, True, False, True)


def build_bass():
    nc = bacc.Bacc()
    xt = nc.declare_dram_parameter("xt", [DIM, T], BF16, isOutput=False)
    wqkvt = nc.declare_dram_parameter("wqkvt", [DIM, 3 * HG], BF16, isOutput=False)
    woutt = nc.declare_dram_parameter("woutt", [HG, DIM], BF16, isOutput=False)
    tri2 = nc.declare_dram_parameter("tri2", [128, 256], BF16, isOutput=False)
    bmf2 = nc.declare_dram_parameter("bmf2", [128, 256], F32, isOutput=False)
    vones = nc.declare_dram_parameter("vones", [128, 8 * 65], BF16, isOutput=False)
    yt = nc.declare_dram_parameter("yt", [DIM, T], F32, isOutput=True)

    with tile.TileContext(nc) as tc:
        _emit(nc, tc, xt, wqkvt, woutt, tri2, bmf2, vones, yt)
    nc.finalize()
    return nc


def _emit(nc, tc, xt, wqkvt, woutt, tri2, bmf2, vones, yt):
    ctx = contextlib.ExitStack()
    with ctx:
        singles = ctx.enter_context(tc.tile_pool(name="singles", bufs=1))
        xpool = ctx.enter_context(tc.tile_pool(name="xpool", bufs=16))
        epoolA = ctx.enter_context(tc.tile_pool(name="epoolA", bufs=3))
        epoolB = ctx.enter_context(tc.tile_pool(name="epoolB", bufs=3))
        apool = ctx.enter_context(tc.tile_pool(name="apool", bufs=1))
        spool = ctx.enter_context(tc.tile_pool(name="spool", bufs=1))
        dpool = ctx.enter_context(tc.tile_pool(name="dpool", bufs=2, space="DRAM"))
        # PSUM budget (8 banks of 2KB/partition):
        #   psq  [128,1024] bufs=2 -> 4 banks (score pairs, double-buffered)
        #   psot [65,512]   bufs=3 -> 3 banks (otA/otB; freed fast by the
        #       SBUF evacuation copy, so 3 bufs cover two head-pairs)
        #   psmm [128,512]  bufs=1 -> 1 bank (warmup + stage 1 + stage 3)
        psq = ctx.enter_context(tc.tile_pool(name="psq", bufs=2, space="PSUM"))
        psot = ctx.enter_context(tc.tile_pool(name="psot", bufs=3, space="PSUM"))
        psmm = ctx.enter_context(tc.tile_pool(name="psmm", bufs=1, space="PSUM"))

        # ---- PE warmup: ~5us of dummy matmuls with no DMA dependency so
        # the HAM clock-gate is at 8/8 when real work arrives.
        wrm = singles.tile([128, 512], BF16, name="wrm")
        nc.vector.memset(wrm, 0.0)
        wps = psmm.tile([128, 512], F32, tag="qkv", name="warm")
        for r in range(12):
            nc.tensor.matmul(wps, lhsT=wrm[:, 0:128], rhs=wrm,
                             start=(r == 0), stop=(r == 11))

        # ---- persistent SBUF tensors ----
        # x tiles for chunk 0 go first on the sync queue; the big weight
        # loads go on the scalar queue so they don't delay stage1(0).
        wq = []
        for k in range(8):
            w = singles.tile([128, 3 * HG], BF16, name=f"wq{k}")
            eng = nc.scalar if k % 2 == 0 else nc.sync
            eng.dma_start(out=w, in_=wqkvt[k * 128:(k + 1) * 128, :])
            wq.append(w)
        wo = []
        for k in range(4):
            w = singles.tile([128, DIM], BF16, name=f"wo{k}")
            nc.scalar.dma_start(out=w, in_=woutt[k * 128:(k + 1) * 128, :])
            wo.append(w)
        trim = singles.tile([128, 256], BF16, name="trim")
        nc.scalar.dma_start(out=trim, in_=tri2[:, :])
        bmf = singles.tile([128, 256], F32, name="bmf")
        nc.scalar.dma_start(out=bmf, in_=bmf2[:, :])

        # QT/KT: 4 tiles [128, 2048] bf16 (2 heads per tile, head-dim major)
        qt = [singles.tile([128, T], BF16, name=f"qt{m}") for m in range(4)]
        kt = [singles.tile([128, T], BF16, name=f"kt{m}") for m in range(4)]
        # V: 16 token-tiles [128, 8*65] bf16 (per head: 64 v-cols + ones col)
        vt = [singles.tile([128, HEADS_PER_CORE * 65], BF16, name=f"vt{t}")
              for t in range(16)]
        for t in range(16):
            nc.scalar.dma_start(out=vt[t], in_=vones[:, :])

        it_ctr = [0]

        def fetch_x(c):
            cs = slice(c * 512, (c + 1) * 512)
            xts = []
            for k in range(8):
                xtile = xpool.tile([128, 512], BF16, tag="xt", name=f"x{c}_{k}")
                nc.sync.dma_start(out=xtile, in_=xt[k * 128:(k + 1) * 128, cs])
                xts.append(xtile)
            return xts

        def stage1(c, xts):
            cs = slice(c * 512, (c + 1) * 512)
            for which, dst in ((0, qt), (1, kt)):
                for m in range(4):
                    ps = psmm.tile([128, 512], F32, tag="qkv", name=f"pq{c}{which}{m}")
                    for k in range(8):
                        nc.tensor.matmul(
                            ps,
                            lhsT=wq[k][:, which * HG + m * 128: which * HG + (m + 1) * 128],
                            rhs=xts[k],
                            start=(k == 0), stop=(k == 7),
                        )
                    if (which * 4 + m) % 2 == 0:
                        nc.scalar.copy(dst[m][:, cs], ps)
                    else:
                        nc.vector.tensor_copy(dst[m][:, cs], ps)
            for i in range(4):
                t = c * 4 + i
                ps = psmm.tile([128, 512], F32, tag="qkv", name=f"pv{t}")
                for k in range(8):
                    nc.tensor.matmul(
                        ps,
                        lhsT=xts[k][:, i * 128:(i + 1) * 128],
                        rhs=wq[k][:, 2 * HG:3 * HG],
                        start=(k == 0), stop=(k == 7),
                    )
                v3 = vt[t].rearrange("p (h d) -> p h d", h=HEADS_PER_CORE)
                nc.scalar.copy(
                    v3[:, :, 0:64],
                    ps.rearrange("p (h d) -> p h d", h=HEADS_PER_CORE))

        def attention(c):
            cs0 = c * 512
            n_kt = KT_PER_CH * (c + 1)
            aot = [apool.tile([128, 512], BF16, tag=f"aot{k}", name=f"aot{c}_{k}")
                   for k in range(4)]
            ots = {}     # hp -> (otA, otB)
            psqs = {}    # (hp, tk) -> psq tile

            def qlo_of(tk):
                j = tk - (n_kt - 4)
                return (max(j, 0) * 128, j)

            def emit_scores(hp, tk):
                qlo, _ = qlo_of(tk)
                q = psq.tile([128, 1024], F32, tag="pair", name=f"s{c}_{hp}_{tk}")
                psqs[(hp, tk)] = q
                for i in range(2):
                    ho = i * 64
                    nc.tensor.matmul(
                        q[:, i * 512 + qlo:(i + 1) * 512],
                        lhsT=kt[hp][ho:ho + 64, tk * 128:(tk + 1) * 128],
                        rhs=qt[hp][ho:ho + 64, cs0 + qlo:cs0 + 512],
                        start=True, stop=True,
                        tile_position=(ho, 0),
                    )

            def emit_ep(hp, tk):
                qlo, j = qlo_of(tk)
                diag = j >= 0
                jq = j * 128
                q = psqs.pop((hp, tk))
                eA = epoolA.tile([128, 512], BF16, tag="e", name=f"ea{c}_{hp}_{tk}")
                eB = epoolB.tile([128, 512], BF16, tag="e", name=f"eb{c}_{hp}_{tk}")
                b_dve = DVE_B_PAT[it_ctr[0] % len(DVE_B_PAT)]
                it_ctr[0] += 1
                # head A: ScalarE exp (+ GpSimd triangle on the diag block)
                nc.scalar.activation(
                    eA[:, qlo:512], q[:, qlo:512],
                    mybir.ActivationFunctionType.Exp, scale=float(SCALE))
                if diag:
                    nc.gpsimd.tensor_mul(
                        eA[:, jq:jq + 128], eA[:, jq:jq + 128], trim[:, 0:128])
                # head B: DVE Schraudolph or ScalarE exp, per pattern
                if b_dve:
                    ei = eB.bitcast(I16)
                    if diag:
                        nc.vector.scalar_tensor_tensor(
                            ei[:, jq:jq + 128], q[:, 512 + jq:512 + jq + 128],
                            SCH_A, bmf[:, 0:128],
                            op0=mybir.AluOpType.mult, op1=mybir.AluOpType.add)
                        if j < 3:
                            nc.vector.tensor_scalar(
                                ei[:, jq + 128:512],
                                q[:, 512 + jq + 128:1024],
                                SCH_A, SCH_B,
                                op0=mybir.AluOpType.mult,
                                op1=mybir.AluOpType.add)
                    else:
                        nc.vector.tensor_scalar(
                            ei, q[:, 512:1024], SCH_A, SCH_B,
                            op0=mybir.AluOpType.mult, op1=mybir.AluOpType.add)
                else:
                    nc.scalar.activation(
                        eB[:, qlo:512], q[:, 512 + qlo:1024],
                        mybir.ActivationFunctionType.Exp, scale=float(SCALE))
                    if diag:
                        nc.gpsimd.tensor_mul(
                            eB[:, jq:jq + 128], eB[:, jq:jq + 128],
                            trim[:, 0:128])
                if tk == 0:
                    ots[hp] = (
                        psot.tile([65, 512], F32, tag="ot", name=f"otA{c}_{hp}"),
                        psot.tile([65, 512], F32, tag="ot", name=f"otB{c}_{hp}"),
                    )
                otA, otB = ots[hp]
                for i, (ot, e) in ((0, (otA, eA)), (1, (otB, eB))):
                    h = 2 * hp + i
                    nc.tensor.matmul(
                        ot[:, qlo:512],
                        lhsT=vt[tk][:, h * 65:h * 65 + 65],
                        rhs=e[:, qlo:512],
                        start=(tk == 0), stop=(tk == n_kt - 1),
                    )
                if tk == n_kt - 1:
                    for i, ot in ((0, otA), (1, otB)):
                        h = 2 * hp + i
                        # evacuate PSUM fast (partition-aligned pieces);
                        # the divide chain then runs from SBUF off-path
                        osb = spool.tile([64, 512], F32, tag="osb", bufs=3,
                                         name=f"ob{c}_{h}")
                        if i == 0:
                            nc.scalar.copy(osb, ot[0:64, :])
                        else:
                            nc.vector.tensor_copy(osb, ot[0:64, :])
                        dn = spool.tile([1, 512], F32, tag="den", bufs=2,
                                        name=f"dn{c}_{h}")
                        nc.vector.tensor_copy(dn, ot[64:65, :])
                        recf = spool.tile([1, 512], F32, tag="recf",
                                          name=f"rf{c}_{h}")
                        nc.vector.reciprocal_approx_fast(recf, dn)
                        dr = dpool.tile([1, 512], F32, tag="dr", name=f"dr{c}_{h}")
                        nc.sync.dma_start(out=dr, in_=recf)
                        bcs = spool.tile([64, 512], F32, tag="bcs", bufs=2,
                                         name=f"bs{c}_{h}")
                        nc.sync.dma_start(out=bcs, in_=dr.to_broadcast((64, 512)))
                        nc.vector.tensor_mul(
                            aot[hp][i * 64:i * 64 + 64, :], osb, bcs)
                    del ots[hp]

            iters = [(hp, tk) for hp in range(4) for tk in range(n_kt)]
            for i, it in enumerate(iters):
                emit_scores(*it)
                if i >= 1:
                    emit_ep(*iters[i - 1])
            emit_ep(*iters[-1])
            return aot

        def stage3(c, aot):
            cs = slice(c * 512, (c + 1) * 512)
            for od in range(8):
                ps = psmm.tile([128, 512], F32, tag="qkv", name=f"py{c}_{od}")
                for k in range(4):
                    nc.tensor.matmul(
                        ps,
                        lhsT=wo[k][:, od * 128:(od + 1) * 128],
                        rhs=aot[k],
                        start=(k == 0), stop=(k == 3),
                    )
                ys = spool.tile([128, 512], F32, tag="ys", bufs=2, name=f"ys{c}_{od}")
                if od % 2 == 0:
                    nc.scalar.copy(ys, ps)
                else:
                    nc.vector.tensor_copy(ys, ps)
                nc.sync.dma_start(out=yt[od * 128:(od + 1) * 128, cs], in_=ys)

        xts = fetch_x(0)
        stage1(0, xts)
        for c in range(NCH):
            if c + 1 < NCH:
                nxts = fetch_x(c + 1)
            aot = attention(c)
            if c + 1 < NCH:
                stage1(c + 1, nxts)
            stage3(c, aot)


_NC_CACHE = None


def _get_nc():
    global _NC_CACHE
    if _NC_CACHE is None:
        _NC_CACHE = build_bass()
    return _NC_CACHE


def make_tri_bm():
    k = np.arange(128)[:, None]
    q = np.arange(128)[None, :]
    keep = (q >= k)
    tri = np.where(keep, 1.0, 0.0).astype(np.float32)
    tri2 = np.concatenate([tri, tri], axis=1).astype(ml_dtypes.bfloat16)
    bm = np.where(keep, SCH_B, SCH_BKILL).astype(np.float32)
    bmf2 = np.concatenate([bm, bm], axis=1)
    return tri2, np.ascontiguousarray(bmf2)


def make_in_maps(x, w_qkv, w_out):
    x = np.asarray(x, dtype=np.float32)
    w_qkv = np.asarray(w_qkv, dtype=np.float32)
    w_out = np.asarray(w_out, dtype=np.float32)
    tri2, bmf2 = make_tri_bm()
    in_maps = []
    for c in range(N_CORES):
        b, g = c // 2, c % 2
        gs = slice(g * HG, (g + 1) * HG)
        wsel = np.concatenate(
            [w_qkv[0 * INNER:][gs], w_qkv[1 * INNER:][gs], w_qkv[2 * INNER:][gs]],
            axis=0)                               # [1536, 1024]
        in_maps.append({
            "xt": np.ascontiguousarray(x[b].T).astype(ml_dtypes.bfloat16),
            "wqkvt": np.ascontiguousarray(wsel.T).astype(ml_dtypes.bfloat16),
            "woutt": np.ascontiguousarray(w_out[:, gs].T).astype(ml_dtypes.bfloat16),
            "tri2": tri2,
            "bmf2": bmf2,
            "vones": np.ones((128, 8 * 65), dtype=ml_dtypes.bfloat16),
        })
    return in_maps


def kernel(x, mask, w_qkv, w_out, **_):
    nc = _get_nc()
    in_maps = make_in_maps(x, w_qkv, w_out)
    res = run_bass_kernel_spmd(nc, in_maps, core_ids=list(range(N_CORES)))
    y = np.zeros((B, T, DIM), dtype=np.float32)
    for c in range(N_CORES):
        y[c // 2] += res.results[c]["yt"].T
    return y


# revision 23
# speedup vs baseline: 1.1721x; 1.1721x over previous
"""Causal self-attention Trainium2 Bass kernel (v3).

Problem: B=4, T=2048, DIM=1024, H=16 heads, head_dim=64 (fp32).
  qkv = x @ w_qkv.T ; per-head causal softmax(q k^T / 8) v ; out @ w_out.T

Sharding (8 cores): core c -> (batch b = c//2, head-group g = c%2 of 8 heads).
Each core computes a partial output y_partial = attn_out_g @ w_out[:, g]^T
for its batch; host sums the two head-group partials per batch.

Structure (per core):
  stage1 (per 512-token chunk): QKV projection; q/k stored head-dim-major
    (qt/kt [128, T], 2 heads per tile), v token-major with a ones column
    per head ([128, 8*65]) so PV also emits the softmax denominator row.
  attention: one flat software pipeline over (head-pair, ktile).  The
    scores matmul stream (2 heads quad-packed via tile_position, K=64)
    runs one iteration ahead of exp+PV.  exp is split BY HEAD: head A on
    ScalarE (native Exp), head B usually on DVE (Schraudolph int16 trick)
    so both halves compute in parallel every iteration and PV never waits
    long.  Causal triangle trimming at 128-col granularity on the 4
    diagonal ktiles (scores/exp/PV only cover q >= j*128).  Diagonal-block
    masking: GpSimd 0/1-triangle multiply (ScalarE halves) or fused into
    the Schraudolph add constant (DVE halves).
  divide: ot PSUM is immediately evacuated to SBUF (frees the PSUM bank in
    ~0.6us instead of holding it through the ~5us divide chain), then
    reciprocal + DRAM-broadcast of 1/den + multiply produce aot.
  stage3: out-projection of the finished chunk.

Startup: PE warmup matmuls (HAM warm by the time stage1 starts) and the
input DMAs split across the two hardware DGE queues (sync + scalar).
"""

import contextlib

import numpy as np
import ml_dtypes

import concourse.bass as bass
import concourse.mybir as mybir
import concourse.tile as tile
from concourse import bacc
from concourse.bass_utils import run_bass_kernel_spmd

B, T, DIM = 4, 2048, 1024
NUM_HEADS, HEAD_DIM = 16, 64
INNER = NUM_HEADS * HEAD_DIM
SCALE = HEAD_DIM ** -0.5

N_CORES = 8
HEADS_PER_CORE = 8
HG = HEADS_PER_CORE * HEAD_DIM  # 512 = inner slice per core
NCH = T // 512                  # 4 token chunks
KT_PER_CH = 4                   # 128-ktok tiles per 512 chunk

F32R = mybir.dt.float32r
F32 = mybir.dt.float32
BF16 = mybir.dt.bfloat16
I16 = mybir.dt.int16

# Schraudolph exp in bf16-bits-as-int16: exp(s*SCALE) ~= bitcast_bf16(
#   int16(s*SCH_A + SCH_B)).  -5.5 centers the mantissa-interp error (+-3%).
LOG2E = 1.4426950408889634
SCH_A = float(SCALE * LOG2E * 128.0)
SCH_B = 127.0 * 128.0 - 5.5
# masked: v = s*SCH_A + SCH_BKILL stays in [~600, ~3500] for any plausible
# score s, so the bf16 bit pattern is a positive subnormal-exponent value
# ~2^-100 (never a negative int16, whose bit pattern would be a bf16 NaN).
SCH_BKILL = 2048.0

# Head-B exp engine per attention iteration: True -> DVE Schraudolph,
# False -> merged into head A's ScalarE activation.
DVE_B_PAT = (True, True, False, True)


def build_bass():
    nc = bacc.Bacc()
    xt = nc.declare_dram_parameter("xt", [DIM, T], BF16, isOutput=False)
    wqkvt = nc.declare_dram_parameter("wqkvt", [DIM, 3 * HG], BF16, isOutput=False)
    woutt = nc.declare_dram_parameter("woutt", [HG, DIM], BF16, isOutput=False)
    tri2 = nc.declare_dram_parameter("tri2", [128, 256], BF16, isOutput=False)
    bmf2 = nc.declare_dram_parameter("bmf2", [128, 256], F32, isOutput=False)
    vones = nc.declare_dram_parameter("vones", [128, 8 * 65], BF16, isOutput=False)
    yt = nc.declare_dram_parameter("yt", [DIM, T], F32, isOutput=True)

    with tile.TileContext(nc) as tc:
        _emit(nc, tc, xt, wqkvt, woutt, tri2, bmf2, vones, yt)
    nc.finalize()
    return nc


def _emit(nc, tc, xt, wqkvt, woutt, tri2, bmf2, vones, yt):
    ctx = contextlib.ExitStack()
    with ctx:
        singles = ctx.enter_context(tc.tile_pool(name="singles", bufs=1))
        xpool = ctx.enter_context(tc.tile_pool(name="xpool", bufs=16))
        epoolA = ctx.enter_context(tc.tile_pool(name="epoolA", bufs=3))
        epoolB = ctx.enter_context(tc.tile_pool(name="epoolB", bufs=3))
        apool = ctx.enter_context(tc.tile_pool(name="apool", bufs=1))
        spool = ctx.enter_context(tc.tile_pool(name="spool", bufs=1))
        dpool = ctx.enter_context(tc.tile_pool(name="dpool", bufs=2, space="DRAM"))
        # PSUM budget (8 banks of 2KB/partition):
        #   psq  [128,1024] bufs=2 -> 4 banks (score pairs, double-buffered)
        #   psot [65,512]   bufs=3 -> 3 banks (otA/otB; freed fast by the
        #       SBUF evacuation copy, so 3 bufs cover two head-pairs)
        #   psmm [128,512]  bufs=1 -> 1 bank (warmup + stage 1 + stage 3)
        psq = ctx.enter_context(tc.tile_pool(name="psq", bufs=2, space="PSUM"))
        psot = ctx.enter_context(tc.tile_pool(name="psot", bufs=3, space="PSUM"))
        psmm = ctx.enter_context(tc.tile_pool(name="psmm", bufs=1, space="PSUM"))

        # ---- PE warmup: ~5us of dummy matmuls with no DMA dependency so
        # the HAM clock-gate is at 8/8 when real work arrives.
        wrm = singles.tile([128, 512], BF16, name="wrm")
        nc.vector.memset(wrm, 0.0)
        wps = psmm.tile([128, 512], F32, tag="qkv", name="warm")
        for r in range(12):
            nc.tensor.matmul(wps, lhsT=wrm[:, 0:128], rhs=wrm,
                             start=(r == 0), stop=(r == 11))

        # ---- persistent SBUF tensors ----
        # x tiles for chunk 0 go first on the sync queue; the big weight
        # loads go on the scalar queue so they don't delay stage1(0).
        wq = []
        for k in range(8):
            w = singles.tile([128, 3 * HG], BF16, name=f"wq{k}")
            nc.scalar.dma_start(out=w, in_=wqkvt[k * 128:(k + 1) * 128, :])
            wq.append(w)
        wo = []
        for k in range(4):
            w = singles.tile([128, DIM], BF16, name=f"wo{k}")
            nc.scalar.dma_start(out=w, in_=woutt[k * 128:(k + 1) * 128, :])
            wo.append(w)
        trim = singles.tile([128, 256], BF16, name="trim")
        nc.scalar.dma_start(out=trim, in_=tri2[:, :])
        bmf = singles.tile([128, 256], F32, name="bmf")
        nc.scalar.dma_start(out=bmf, in_=bmf2[:, :])

        # QT/KT: 4 tiles [128, 2048] bf16 (2 heads per tile, head-dim major)
        qt = [singles.tile([128, T], BF16, name=f"qt{m}") for m in range(4)]
        kt = [singles.tile([128, T], BF16, name=f"kt{m}") for m in range(4)]
        # V: 16 token-tiles [128, 8*65] bf16 (per head: 64 v-cols + ones col)
        vt = [singles.tile([128, HEADS_PER_CORE * 65], BF16, name=f"vt{t}")
              for t in range(16)]
        for t in range(16):
            nc.scalar.dma_start(out=vt[t], in_=vones[:, :])

        it_ctr = [0]

        def fetch_x(c):
            cs = slice(c * 512, (c + 1) * 512)
            xts = []
            for k in range(8):
                xtile = xpool.tile([128, 512], BF16, tag="xt", name=f"x{c}_{k}")
                nc.sync.dma_start(out=xtile, in_=xt[k * 128:(k + 1) * 128, cs])
                xts.append(xtile)
            return xts

        def stage1(c, xts):
            cs = slice(c * 512, (c + 1) * 512)
            for which, dst in ((0, qt), (1, kt)):
                for m in range(4):
                    ps = psmm.tile([128, 512], F32, tag="qkv", name=f"pq{c}{which}{m}")
                    for k in range(8):
                        nc.tensor.matmul(
                            ps,
                            lhsT=wq[k][:, which * HG + m * 128: which * HG + (m + 1) * 128],
                            rhs=xts[k],
                            start=(k == 0), stop=(k == 7),
                        )
                    if (which * 4 + m) % 2 == 0:
                        nc.scalar.copy(dst[m][:, cs], ps)
                    else:
                        nc.vector.tensor_copy(dst[m][:, cs], ps)
            for i in range(4):
                t = c * 4 + i
                ps = psmm.tile([128, 512], F32, tag="qkv", name=f"pv{t}")
                for k in range(8):
                    nc.tensor.matmul(
                        ps,
                        lhsT=xts[k][:, i * 128:(i + 1) * 128],
                        rhs=wq[k][:, 2 * HG:3 * HG],
                        start=(k == 0), stop=(k == 7),
                    )
                v3 = vt[t].rearrange("p (h d) -> p h d", h=HEADS_PER_CORE)
                nc.scalar.copy(
                    v3[:, :, 0:64],
                    ps.rearrange("p (h d) -> p h d", h=HEADS_PER_CORE))

        def attention(c):
            cs0 = c * 512
            n_kt = KT_PER_CH * (c + 1)
            aot = [apool.tile([128, 512], BF16, tag=f"aot{k}", name=f"aot{c}_{k}")
                   for k in range(4)]
            ots = {}     # hp -> (otA, otB)
            psqs = {}    # (hp, tk) -> psq tile

            def qlo_of(tk):
                j = tk - (n_kt - 4)
                return (max(j, 0) * 128, j)

            def emit_scores(hp, tk):
                qlo, _ = qlo_of(tk)
                q = psq.tile([128, 1024], F32, tag="pair", name=f"s{c}_{hp}_{tk}")
                psqs[(hp, tk)] = q
                for i in range(2):
                    ho = i * 64
                    nc.tensor.matmul(
                        q[:, i * 512 + qlo:(i + 1) * 512],
                        lhsT=kt[hp][ho:ho + 64, tk * 128:(tk + 1) * 128],
                        rhs=qt[hp][ho:ho + 64, cs0 + qlo:cs0 + 512],
                        start=True, stop=True,
                        tile_position=(ho, 0),
                    )

            def emit_ep(hp, tk):
                qlo, j = qlo_of(tk)
                diag = j >= 0
                jq = j * 128
                q = psqs.pop((hp, tk))
                eA = epoolA.tile([128, 512], BF16, tag="e", name=f"ea{c}_{hp}_{tk}")
                eB = epoolB.tile([128, 512], BF16, tag="e", name=f"eb{c}_{hp}_{tk}")
                b_dve = DVE_B_PAT[it_ctr[0] % len(DVE_B_PAT)]
                it_ctr[0] += 1
                # head A: ScalarE exp (+ GpSimd triangle on the diag block)
                nc.scalar.activation(
                    eA[:, qlo:512], q[:, qlo:512],
                    mybir.ActivationFunctionType.Exp, scale=float(SCALE))
                if diag:
                    nc.gpsimd.tensor_mul(
                        eA[:, jq:jq + 128], eA[:, jq:jq + 128], trim[:, 0:128])
                # head B: DVE Schraudolph or ScalarE exp, per pattern
                if b_dve:
                    ei = eB.bitcast(I16)
                    if diag:
                        nc.vector.scalar_tensor_tensor(
                            ei[:, jq:jq + 128], q[:, 512 + jq:512 + jq + 128],
                            SCH_A, bmf[:, 0:128],
                            op0=mybir.AluOpType.mult, op1=mybir.AluOpType.add)
                        if j < 3:
                            nc.vector.tensor_scalar(
                                ei[:, jq + 128:512],
                                q[:, 512 + jq + 128:1024],
                                SCH_A, SCH_B,
                                op0=mybir.AluOpType.mult,
                                op1=mybir.AluOpType.add)
                    else:
                        nc.vector.tensor_scalar(
                            ei, q[:, 512:1024], SCH_A, SCH_B,
                            op0=mybir.AluOpType.mult, op1=mybir.AluOpType.add)
                else:
                    nc.scalar.activation(
                        eB[:, qlo:512], q[:, 512 + qlo:1024],
                        mybir.ActivationFunctionType.Exp, scale=float(SCALE))
                    if diag:
                        nc.gpsimd.tensor_mul(
                            eB[:, jq:jq + 128], eB[:, jq:jq + 128],
                            trim[:, 0:128])
                if tk == 0:
                    ots[hp] = (
                        psot.tile([65, 512], F32, tag="ot", name=f"otA{c}_{hp}"),
                        psot.tile([65, 512], F32, tag="ot", name=f"otB{c}_{hp}"),
                    )
                otA, otB = ots[hp]
                for i, (ot, e) in ((0, (otA, eA)), (1, (otB, eB))):
                    h = 2 * hp + i
                    nc.tensor.matmul(
                        ot[:, qlo:512],
                        lhsT=vt[tk][:, h * 65:h * 65 + 65],
                        rhs=e[:, qlo:512],
                        start=(tk == 0), stop=(tk == n_kt - 1),
                    )
                if tk == n_kt - 1:
                    for i, ot in ((0, otA), (1, otB)):
                        h = 2 * hp + i
                        # evacuate PSUM fast (partition-aligned pieces);
                        # the divide chain then runs from SBUF off-path
                        osb = spool.tile([64, 512], F32, tag="osb", bufs=3,
                                         name=f"ob{c}_{h}")
                        if i == 0:
                            nc.scalar.copy(osb, ot[0:64, :])
                        else:
                            nc.vector.tensor_copy(osb, ot[0:64, :])
                        dn = spool.tile([1, 512], F32, tag="den", bufs=2,
                                        name=f"dn{c}_{h}")
                        nc.vector.tensor_copy(dn, ot[64:65, :])
                        recf = spool.tile([1, 512], F32, tag="recf",
                                          name=f"rf{c}_{h}")
                        nc.vector.reciprocal_approx_fast(recf, dn)
                        dr = dpool.tile([1, 512], F32, tag="dr", name=f"dr{c}_{h}")
                        nc.sync.dma_start(out=dr, in_=recf)
                        bcs = spool.tile([64, 512], F32, tag="bcs", bufs=2,
                                         name=f"bs{c}_{h}")
                        nc.sync.dma_start(out=bcs, in_=dr.to_broadcast((64, 512)))
                        nc.vector.tensor_mul(
                            aot[hp][i * 64:i * 64 + 64, :], osb, bcs)
                    del ots[hp]

            iters = [(hp, tk) for hp in range(4) for tk in range(n_kt)]
            for i, it in enumerate(iters):
                emit_scores(*it)
                if i >= 1:
                    emit_ep(*iters[i - 1])
            emit_ep(*iters[-1])
            return aot

        def stage3(c, aot):
            cs = slice(c * 512, (c + 1) * 512)
            for od in range(8):
                ps = psmm.tile([128, 512], F32, tag="qkv", name=f"py{c}_{od}")
                for k in range(4):
                    nc.tensor.matmul(
                        ps,
                        lhsT=wo[k][:, od * 128:(od + 1) * 128],
                        rhs=aot[k],
                        start=(k == 0), stop=(k == 3),
                    )
                ys = spool.tile([128, 512], F32, tag="ys", bufs=2, name=f"ys{c}_{od}")
                if od % 2 == 0:
                    nc.scalar.copy(ys, ps)
                else:
                    nc.vector.tensor_copy(ys, ps)
                nc.sync.dma_start(out=yt[od * 128:(od + 1) * 128, cs], in_=ys)

        xts = fetch_x(0)
        stage1(0, xts)
        for c in range(NCH):
            if c + 1 < NCH:
                nxts = fetch_x(c + 1)
            aot = attention(c)
            if c + 1 < NCH:
                stage1(c + 1, nxts)
            stage3(c, aot)


_NC_CACHE = None


def _get_nc():
    global _NC_CACHE
    if _NC_CACHE is None:
        _NC_CACHE = build_bass()
    return _NC_CACHE


def make_tri_bm():
    k = np.arange(128)[:, None]
    q = np.arange(128)[None, :]
    keep = (q >= k)
    tri = np.where(keep, 1.0, 0.0).astype(np.float32)
    tri2 = np.concatenate([tri, tri], axis=1).astype(ml_dtypes.bfloat16)
    bm = np.where(keep, SCH_B, SCH_BKILL).astype(np.float32)
    bmf2 = np.concatenate([bm, bm], axis=1)
    return tri2, np.ascontiguousarray(bmf2)


def make_in_maps(x, w_qkv, w_out):
    x = np.asarray(x, dtype=np.float32)
    w_qkv = np.asarray(w_qkv, dtype=np.float32)
    w_out = np.asarray(w_out, dtype=np.float32)
    tri2, bmf2 = make_tri_bm()
    in_maps = []
    for c in range(N_CORES):
        b, g = c // 2, c % 2
        gs = slice(g * HG, (g + 1) * HG)
        wsel = np.concatenate(
            [w_qkv[0 * INNER:][gs], w_qkv[1 * INNER:][gs], w_qkv[2 * INNER:][gs]],
            axis=0)                               # [1536, 1024]
        in_maps.append({
            "xt": np.ascontiguousarray(x[b].T).astype(ml_dtypes.bfloat16),
            "wqkvt": np.ascontiguousarray(wsel.T).astype(ml_dtypes.bfloat16),
            "woutt": np.ascontiguousarray(w_out[:, gs].T).astype(ml_dtypes.bfloat16),
            "tri2": tri2,
            "bmf2": bmf2,
            "vones": np.ones((128, 8 * 65), dtype=ml_dtypes.bfloat16),
        })
    return in_maps


def kernel(x, mask, w_qkv, w_out, **_):
    nc = _get_nc()
    in_maps = make_in_maps(x, w_qkv, w_out)
    res = run_bass_kernel_spmd(nc, in_maps, core_ids=list(range(N_CORES)))
    y = np.zeros((B, T, DIM), dtype=np.float32)
    for c in range(N_CORES):
        y[c // 2] += res.results[c]["yt"].T
    return y
